# revision 14
# baseline (speedup 1.0000x reference)
"""Trainium2 Bass kernel for nn_MultiHeadNetWithNestedHistory.

Math (per batch b):
    k        = keys_b @ key_w.T                      [S, D]
    v        = keys_b @ value_w.T                    [S, D]
    qrow     = q_flat + (cond_w @ ctx_b + cond_b)    [Q, D]
    logits   = (qrow @ key_w) @ keys_b.T * scale * inv_t[q]   (key_w folded left)
    logits  += alibi/mask bias
    w        = exp(logits)          (logits bounded above by ~1, no row-max pass)
    t        = (w/sum) @ keys_b                      [Q, D]
    out      = t @ value_w.T                         [Q, D]

Host folds all param-only products:
    W3aug[c, (q,d)] : maps context (+ ones row for the bias/base term) directly
                      to the scaled qk^T, so one on-chip matmul produces
                      qk^T[d,(b,q)] for all local batches.
Sharding: pure data-parallel over batch, 64 batches per core on 8 cores.
All matmuls run as float32r (TF32-like, ~1e-4 rel err, 4x faster than fp32).
"""

import json as _json

import numpy as np

B, S, D = 512, 512, 256
NT, NH, COND = 8, 4, 64
Q = NT * NH  # 32
NCORES = 8
BL = B // NCORES       # 64 batches per core
NG = BL // 4           # 16 groups of 4 batches
CA = COND + 1          # context augmented with ones row


def _split_sync_waits(bir_bytes):
    """This walrus build accepts at most ONE sync-wait per instruction.
    Hoist extra waits onto NoOps inserted before, on the same engine."""
    d = _json.loads(bir_bytes)
    ctr = 0
    for fn in d.get("functions", []):
        for blk in fn.get("blocks", []):
            insts = blk.get("instructions")
            if not insts:
                continue
            new_insts = []
            for ins in insts:
                si = ins.get("sync_info")
                ow = (si or {}).get("on_wait") or []
                if len(ow) > 1:
                    for w in ow[:-1]:
                        ctr += 1
                        nop = {
                            "engine": ins["engine"],
                            "ins": [],
                            "outs": [],
                            "name": f"I-splitw-{ctr}",
                            "opcode": "NoOp",
                            "sync_info": {"on_update": [], "on_wait": [w]},
                            "text_hint": "split_wait",
                        }
                        if "debug" in ins:
                            nop["debug"] = ins["debug"]
                        new_insts.append(nop)
                    si["on_wait"] = [ow[-1]]
                new_insts.append(ins)
            blk["instructions"] = new_insts
    return _json.dumps(d).encode()


def _install_patches():
    import concourse.bass as bass
    if getattr(bass.Bass, "_split_waits_installed", False):
        return
    orig = bass.Bass.to_json_bytes

    def to_json_bytes(self, *a, **k):
        return _split_sync_waits(orig(self, *a, **k))

    bass.Bass.to_json_bytes = to_json_bytes
    bass.Bass._split_waits_installed = True


def _build_nc():
    import concourse.bass as bass
    import concourse.mybir as mybir
    from concourse.tile import TileContext

    f32, f32r = mybir.dt.float32, mybir.dt.float32r
    AL = mybir.AluOpType
    AF = mybir.ActivationFunctionType
    AX = mybir.AxisListType

    nc = bass.Bass()
    keysD = nc.declare_dram_parameter("keys", [BL, S, D], f32r, isOutput=False)
    maskD = nc.declare_dram_parameter("maskf", [BL, S], mybir.dt.uint8, isOutput=False)
    ctxD = nc.declare_dram_parameter("ctxaug", [CA, BL], f32r, isOutput=False)
    w3D = nc.declare_dram_parameter("w3aug", [CA, Q * D], f32r, isOutput=False)
    vwD = nc.declare_dram_parameter("vwt", [D, D], f32r, isOutput=False)
    idD = nc.declare_dram_parameter("ident", [128, 128], f32r, isOutput=False)
    nsD = nc.declare_dram_parameter("negslope", [128, 1], f32, isOutput=False)
    selD = nc.declare_dram_parameter("selres", [BL, 128], f32r, isOutput=False)
    indD = nc.declare_dram_parameter("indpad", [BL, BL], f32, isOutput=False)
    outD = nc.declare_dram_parameter("out", [BL * Q, D], f32, isOutput=True)

    def mk_ap(base_ap, offset_elems, ap_list):
        return bass.AP(tensor=base_ap.tensor, offset=base_ap.offset + offset_elems,
                       ap=ap_list)

    with TileContext(nc) as tc:
        with (
            tc.tile_pool(name="const", bufs=1) as cpool,
            tc.tile_pool(name="store", bufs=1) as spool,
            tc.tile_pool(name="keys", bufs=8) as kpool,
            tc.tile_pool(name="kt", bufs=4) as ktpool,
            tc.tile_pool(name="soft", bufs=2) as fpool,
            tc.tile_pool(name="gsmall", bufs=2) as gpool,
            tc.tile_pool(name="psK", bufs=2, space="PSUM") as psK,
            tc.tile_pool(name="psA", bufs=2, space="PSUM") as psA,
            tc.tile_pool(name="psS", bufs=4, space="PSUM") as psS,
        ):
            # ---------- constants / params ----------
            w3 = cpool.tile([CA, Q * D], f32r, tag="w3")
            nc.sync.dma_start(out=w3[:], in_=w3D[:])
            ctx = cpool.tile([CA, BL], f32r, tag="ctx")
            nc.sync.dma_start(out=ctx[:], in_=ctxD[:])
            idt = cpool.tile([128, 128], f32r, tag="idt")
            nc.sync.dma_start(out=idt[:], in_=idD[:])
            nsl = cpool.tile([128, 1], f32, tag="nsl")
            nc.sync.dma_start(out=nsl[:], in_=nsD[:])
            # value_w.T as [128, (chunk, e)]
            vw = cpool.tile([128, 2, D], f32r, tag="vw")
            vw_src = mk_ap(vwD[:], 0, [[D, 128], [128 * D, 2], [1, D]])
            nc.sync.dma_start(out=vw[:], in_=vw_src)
            # g0[s] = -1 - s
            g0 = cpool.tile([128, S], f32, tag="g0")
            nc.gpsimd.iota(g0[:], pattern=[[-1, S]], base=-1, channel_multiplier=0,
                           allow_small_or_imprecise_dtypes=True)
            # n_real for every local batch (mask is a prefix mask), broadcast to
            # the per-group (b,q)-partition layout via a one-hot matmul:
            # NR[p, g] = nall[4g + p//32]
            selres = cpool.tile([BL, 128], f32r, tag="selres")
            nc.sync.dma_start(out=selres[:], in_=selD[:])
            ind = cpool.tile([BL, BL], f32, tag="ind")
            nc.sync.dma_start(out=ind[:], in_=indD[:])
            mall = cpool.tile([BL, S], f32, tag="mall")
            nc.gpsimd.dma_start(out=mall[:], in_=maskD[:])
            nall = cpool.tile([BL, 1], f32, tag="nall")
            nc.vector.tensor_reduce(out=nall[:], in_=mall[:], axis=AX.X, op=AL.add)
            nmat = cpool.tile([BL, BL], f32r, tag="nmat")
            nc.vector.tensor_scalar(out=nmat[:], in0=ind[:], scalar1=nall[:],
                                    scalar2=None, op0=AL.mult)
            nrP = psS.tile([128, BL], f32, tag="s")
            nc.tensor.matmul(nrP[:], selres[:], nmat[:], start=True, stop=True)
            NR = cpool.tile([128, BL], f32, tag="NR")
            nc.vector.tensor_copy(NR[:], nrP[:])

            # ---------- qk^T for all local batches ----------
            # qkT[dh][d, b*32+q] = scaled qk^T
            qkT = [spool.tile([128, BL * Q], f32r, tag=f"qkT{dh}", name=f"qkT{dh}")
                   for dh in range(2)]
            for q in range(Q):
                for dh in range(2):
                    qkP = psS.tile([128, BL], f32, tag="s")
                    nc.tensor.matmul(qkP[:], w3[:, q * D + dh * 128: q * D + dh * 128 + 128],
                                     ctx[:], start=True, stop=True)
                    dst = mk_ap(qkT[dh], q, [qkT[dh].ap[0], [Q, BL]])
                    nc.scalar.copy(dst, qkP[:])

            # ---------- main loop ----------
            for g in range(NG):
                nreal = NR[:, g:g + 1]
                # u = n-1-s ; bias = -slope * max(u, -1e10*u - 5e9)
                u = gpool.tile([128, S], f32, tag="u")
                nc.vector.tensor_scalar(out=u[:], in0=g0[:], scalar1=nreal[:],
                                        scalar2=None, op0=AL.add)
                t1 = gpool.tile([128, S], f32, tag="t1")
                nc.vector.tensor_scalar(out=t1[:], in0=u[:], scalar1=-1e10,
                                        scalar2=-5e9, op0=AL.mult, op1=AL.add)
                fb = gpool.tile([128, S], f32, tag="fb")
                nc.vector.tensor_tensor(out=fb[:], in0=u[:], in1=t1[:], op=AL.max)
                bias = gpool.tile([128, S], f32, tag="bias")
                nc.vector.tensor_scalar(out=bias[:], in0=fb[:], scalar1=nsl[:],
                                        scalar2=None, op0=AL.mult)

                fA = fpool.tile([128, S], f32, tag="fA")
                kN = []
                for j in range(4):
                    b = 4 * g + j
                    kn = kpool.tile([128, 4, D], f32r, tag="kn")
                    ksrc = mk_ap(keysD[:], b * S * D, [[D, 128], [128 * D, 4], [1, D]])
                    nc.sync.dma_start(out=kn[:], in_=ksrc)
                    kN.append(kn)
                    kT = []
                    for c in range(2):
                        kTp = psK.tile([128, S], f32r, tag="kTp")
                        for i in range(4):
                            nc.tensor.transpose(kTp[:, i * 128:(i + 1) * 128],
                                                kn[:, i, c * 128:(c + 1) * 128], idt[:])
                        kt = ktpool.tile([128, S], f32r, tag="kt")
                        nc.scalar.copy(kt[:], kTp[:])
                        kT.append(kt)
                    attnP = psA.tile([32, S], f32, tag="attnP")
                    nc.tensor.matmul(attnP[:], qkT[0][:, b * Q: b * Q + Q], kT[0][:],
                                     start=True, stop=False)
                    nc.tensor.matmul(attnP[:], qkT[1][:, b * Q: b * Q + Q], kT[1][:],
                                     start=False, stop=True)
                    nc.vector.tensor_tensor(out=fA[32 * j:32 * (j + 1)], in0=attnP[:],
                                            in1=bias[32 * j:32 * (j + 1)], op=AL.add)

                # softmax numerator + row sums (logits bounded above -> no max pass)
                w = fpool.tile([128, S], f32r, tag="w")
                ssum = gpool.tile([128, 1], f32, tag="ssum")
                nc.scalar.activation(w[:], fA[:], AF.Exp, bias=0.0, scale=1.0,
                                     accum_out=ssum[:])
                rcp = gpool.tile([128, 1], f32, tag="rcp")
                nc.vector.reciprocal(rcp[:], ssum[:])

                # attn^T[(s), (b,q)]
                aT = fpool.tile([128, 4, 128], f32r, tag="aT")
                for i in range(4):
                    aTp = psS.tile([128, 128], f32r, tag="s")
                    nc.tensor.transpose(aTp[:], w[:, i * 128:(i + 1) * 128], idt[:])
                    nc.vector.tensor_copy(aT[:, i, :], aTp[:])

                # t = w @ keys   (contract s; keys natural layout)
                tg = gpool.tile([128, D], f32r, tag="tg")
                for j in range(4):
                    tP = psS.tile([32, D], f32, tag="s")
                    for i in range(4):
                        nc.tensor.matmul(tP[:], aT[:, i, 32 * j:32 * (j + 1)],
                                         kN[j][:, i, :], start=(i == 0), stop=(i == 3))
                    nc.scalar.copy(tg[32 * j:32 * (j + 1)], tP[:])

                # t^T[(d), (b,q)]
                tT = gpool.tile([128, 2, 128], f32r, tag="tT")
                for c in range(2):
                    tTp = psS.tile([128, 128], f32r, tag="s")
                    nc.tensor.transpose(tTp[:], tg[:, c * 128:(c + 1) * 128], idt[:])
                    nc.vector.tensor_copy(tT[:, c, :], tTp[:])

                # out = t @ value_w.T, normalized by 1/sum during eviction
                og = gpool.tile([128, D], f32, tag="og")
                for j in range(4):
                    oP = psS.tile([32, D], f32, tag="s")
                    for c in range(2):
                        nc.tensor.matmul(oP[:], tT[:, c, 32 * j:32 * (j + 1)],
                                         vw[:, c, :], start=(c == 0), stop=(c == 1))
                    nc.vector.tensor_scalar(out=og[32 * j:32 * (j + 1)], in0=oP[:],
                                            scalar1=rcp[32 * j:32 * (j + 1)],
                                            scalar2=None, op0=AL.mult)
                nc.sync.dma_start(out=outD[128 * g:128 * (g + 1), :], in_=og[:])
    return nc


_NC_CACHE = {}


def _get_nc():
    if "nc" not in _NC_CACHE:
        _install_patches()
        _NC_CACHE["nc"] = _build_nc()
    return _NC_CACHE["nc"]


def _alibi_slopes(n_heads):
    return np.array([2.0 ** (-8.0 * (h + 1) / n_heads) for h in range(n_heads)],
                    dtype=np.float32)


def kernel(keys, mask, context, queries, key_w, value_w, log_temperature,
           cond_w, cond_b, _trace=False):
    keys = np.asarray(keys, dtype=np.float32)
    mask = np.asarray(mask)
    context = np.asarray(context, dtype=np.float32)
    queries = np.asarray(queries, dtype=np.float32)
    key_w = np.asarray(key_w, dtype=np.float32)
    value_w = np.asarray(value_w, dtype=np.float32)
    log_temperature = np.asarray(log_temperature, dtype=np.float32)
    cond_w = np.asarray(cond_w, dtype=np.float32)
    cond_b = np.asarray(cond_b, dtype=np.float32)

    # ---- host folding of parameter-only products ----
    s_q = (np.repeat(np.exp(-log_temperature), NH) * (D ** -0.5)).astype(np.float32)
    q_flat = queries.reshape(Q, D)
    qbase = (q_flat + cond_b.reshape(Q, D)) @ key_w          # [Q, D]
    W3 = np.einsum("qec,ed->qdc", cond_w.reshape(Q, D, COND), key_w)  # [Q, D, C]
    W3s = (W3 * s_q[:, None, None]).astype(np.float32)
    qbase_s = (qbase * s_q[:, None]).astype(np.float32)
    w3aug = np.empty((CA, Q * D), dtype=np.float32)
    w3aug[:COND] = W3s.transpose(2, 0, 1).reshape(COND, Q * D)
    w3aug[COND] = qbase_s.reshape(Q * D)

    vwt = np.ascontiguousarray(value_w.T)                    # [d, e]
    ident = np.eye(128, dtype=np.float32)
    slopes_q = np.tile(_alibi_slopes(NH), NT)                # [32]
    negslope = (-np.tile(slopes_q, 4)[:, None]).astype(np.float32)  # [128,1]
    maskf = mask.astype(np.uint8)
    # selres[b, p] = 1 iff b % 4 == p//32 ; indpad[b, g] = 1 iff b//4 == g (g<16)
    pidx = np.arange(128)
    bidx = np.arange(BL)
    selres = (bidx[:, None] % 4 == pidx[None, :] // 32).astype(np.float32)
    indpad = np.zeros((BL, BL), dtype=np.float32)
    indpad[bidx, bidx // 4] = 1.0

    in_maps = []
    for c in range(NCORES):
        sl = slice(c * BL, (c + 1) * BL)
        ctxaug = np.empty((CA, BL), dtype=np.float32)
        ctxaug[:COND] = context[sl].T
        ctxaug[COND] = 1.0
        in_maps.append({
            "keys": np.ascontiguousarray(keys[sl]),
            "maskf": np.ascontiguousarray(maskf[sl]),
            "ctxaug": ctxaug,
            "w3aug": w3aug,
            "vwt": vwt,
            "ident": ident,
            "negslope": negslope,
            "selres": selres,
            "indpad": indpad,
        })

    from concourse.bass_utils import run_bass_kernel_spmd
    nc = _get_nc()
    kwargs = {}
    if _trace:
        import tempfile
        kwargs = dict(trace=True, tmpdir=tempfile.mkdtemp(prefix="bass_trace_"))
    res = run_bass_kernel_spmd(nc, in_maps, list(range(NCORES)), **kwargs)

    out = np.empty((B, NT, NH * D), dtype=np.float32)
    for c in range(NCORES):
        out[c * BL:(c + 1) * BL] = res.results[c]["out"].reshape(BL, NT, NH * D)
    if _trace:
        return out, res
    return out
